# revision 21
# baseline (speedup 1.0000x reference)
"""Causal self-attention (B=4, T=2048, C=1024, H=16) on 8 trn2 NeuronCores.

Sharding: core = 2*b + g  (b = batch 0..3, g = head-group 0..1).
Each core handles 1 batch and 8 heads and returns a partial projection
output [T, C]; the host sums the two head-group partials per batch.

v3: flash-style q-superstep ordering + transposed-Y attention + fp8
DoubleRow 3-term-compensated projections (QK, V, P4) + deficit-scheduled
PE filler interleave so projection/P4 matmuls hide the Act-engine-bound
exp phase.

Per-superstep i (512 q):
  P1  project Q,K (feature-major [128j, t]) and V (token-major [t, j]) for
      this t-range via fp8 DR matmuls:  x_hi@W_hi + x_lo@W_hi + x_hi@W_lo,
      each term 4 DoubleRow matmuls (256-deep contraction, 0.5 cyc/row).
  P3  per (head, kt<=4i+3): S_T = K.T Q [128k, 512q] (bf16), E = exp(S/8)
      on Act, tri-mask diag blocks on DVE; per 128-q subtile accumulate
      y[q, 64d | rowsum] += E_blk.T @ [V|1] (transposed-Y: 65 rows/block).
      S runs one kt ahead of Y; fillers (future proj / deferred P4) are
      drained whenever the Act-minus-PE deficit estimate goes positive.
      Normalize: one DVE reciprocal [128,4] per (head, step) + scalar-mul.
  P5  (deferred, as filler) transpose y [t,j]->[j,t] on PE, split yT into
      fp8 hi/lo on DVE, P4 out = yT.T @ wp via fp8 DR matmuls, stream out.
"""

import numpy as np
import ml_dtypes

B, T, C = 4, 2048, 1024
H_LOC = 8          # heads per core
HD = 64            # head dim
N_CORES = 8
QT = 512           # q super-step width
NST = T // QT      # 4
NCT = C // 128     # 8 contraction chunks for qkv

bf16 = ml_dtypes.bfloat16
f8 = ml_dtypes.float8_e4m3

# fp8 hi/lo splits are pre-scaled so the lo residuals stay in e4m3's normal
# range (unscaled weight residuals ~1e-3 fall below the 2^-9 min subnormal
# and flush to zero, killing the compensation).  Products carry SX*SW=512;
# PSUM evacuations divide it back out.
SX = 8.0           # x pre-scale
SW = 64.0          # weight pre-scale
SY = 8.0           # on-device y pre-scale for the P4 hi/lo split

PE_NS = 0.417      # ns per output row at full pstate
ACT_NS = 0.833
ACT_OVH = 185.0

_CACHE = {}


def _build():
    import concourse.bacc as bacc
    import concourse.tile as tile
    import concourse.mybir as mybir
    from contextlib import ExitStack

    f32 = mybir.dt.float32
    b16 = mybir.dt.bfloat16
    fp8 = mybir.dt.float8e4
    EXP = mybir.ActivationFunctionType.Exp
    DR = mybir.MatmulPerfMode.DoubleRow

    nc = bacc.Bacc("TRN2", target_bir_lowering=False, debug=False)

    xh_d = nc.dram_tensor("xh", [C, T], fp8, kind="ExternalInput").ap()
    xl_d = nc.dram_tensor("xl", [C, T], fp8, kind="ExternalInput").ap()
    wqkh_d = nc.dram_tensor("wqkh", [C, 1024], fp8, kind="ExternalInput").ap()
    wqkl_d = nc.dram_tensor("wqkl", [C, 1024], fp8, kind="ExternalInput").ap()
    wvh_d = nc.dram_tensor("wvh", [C, 512], fp8, kind="ExternalInput").ap()
    wvl_d = nc.dram_tensor("wvl", [C, 512], fp8, kind="ExternalInput").ap()
    wph_d = nc.dram_tensor("wph", [512, C], fp8, kind="ExternalInput").ap()
    wpl_d = nc.dram_tensor("wpl", [512, C], fp8, kind="ExternalInput").ap()
    tri_d = nc.dram_tensor("tri", [128, 128], b16, kind="ExternalInput").ap()
    ident_d = nc.dram_tensor("ident", [128, 128], b16, kind="ExternalInput").ap()
    out_d = nc.dram_tensor("out", [T, C], f32, kind="ExternalOutput").ap()

    with tile.TileContext(nc) as tc:
        with ExitStack() as ctx:
            pers = ctx.enter_context(tc.tile_pool(name="pers", bufs=1))
            xh_sb = pers.tile([128, NCT, T], fp8)
            xl_sb = pers.tile([128, NCT, T], fp8)
            wqkh_sb = pers.tile([128, NCT, 1024], fp8)
            wqkl_sb = pers.tile([128, NCT, 1024], fp8)
            wvh_sb = pers.tile([128, NCT, 512], fp8)
            wvl_sb = pers.tile([128, NCT, 512], fp8)
            wph_sb = pers.tile([128, 4, C], fp8)
            wpl_sb = pers.tile([128, 4, C], fp8)
            tri_sb = pers.tile([128, 128], b16)
            id_sb = pers.tile([128, 128], b16)
            qk_sb = pers.tile([128, 8, T], b16)   # jt 0..3 Q pairs, 4..7 K pairs
            vt_sb = pers.tile([128, T // 128, H_LOC, HD + 2], b16)  # ones col @64
            y_tiles = [pers.tile([128, H_LOC, HD], b16, name=f"y{t}")
                       for t in range(T // 128)]

            # q,k,v are kept scaled by SX*SW in SBUF (evacuated via plain
            # Pool-engine DMA copies, which cannot scale); the descale is
            # absorbed into the exp scale (S carries (SX*SW)^2) and into the
            # softmax normalization (ones column = SX*SW so rowsums carry the
            # same factor as the y columns).
            nc.gpsimd.memset(vt_sb[:, :, :, HD], SX * SW)

            xh_r = xh_d.rearrange("(a p) t -> p a t", p=128)
            xl_r = xl_d.rearrange("(a p) t -> p a t", p=128)
            wqkh_r = wqkh_d.rearrange("(a p) j -> p a j", p=128)
            wqkl_r = wqkl_d.rearrange("(a p) j -> p a j", p=128)
            wvh_r = wvh_d.rearrange("(a p) j -> p a j", p=128)
            wvl_r = wvl_d.rearrange("(a p) j -> p a j", p=128)
            wph_r = wph_d.rearrange("(c p) j -> p c j", p=128)
            wpl_r = wpl_d.rearrange("(c p) j -> p c j", p=128)

            # step-0 x and pair-0 wqk columns first so attention starts early
            nc.sync.dma_start(xh_sb[:, :, 0:QT], xh_r[:, :, 0:QT])
            for jt in (0, 4):
                c = slice(jt * 128, (jt + 1) * 128)
                nc.sync.dma_start(wqkh_sb[:, :, c], wqkh_r[:, :, c])
                nc.sync.dma_start(wqkl_sb[:, :, c], wqkl_r[:, :, c])
            nc.sync.dma_start(xl_sb[:, :, 0:QT], xl_r[:, :, 0:QT])
            nc.sync.dma_start(tri_sb, tri_d)
            nc.sync.dma_start(id_sb, ident_d)
            for jt in (1, 5, 2, 6, 3, 7):
                c = slice(jt * 128, (jt + 1) * 128)
                nc.sync.dma_start(wqkh_sb[:, :, c], wqkh_r[:, :, c])
                nc.sync.dma_start(wqkl_sb[:, :, c], wqkl_r[:, :, c])
                if jt == 5:
                    nc.sync.dma_start(wvh_sb, wvh_r)
                    nc.sync.dma_start(wvl_sb, wvl_r)
            for i in range(1, NST):
                s = slice(i * QT, (i + 1) * QT)
                nc.sync.dma_start(xh_sb[:, :, s], xh_r[:, :, s])
                nc.sync.dma_start(xl_sb[:, :, s], xl_r[:, :, s])
            nc.sync.dma_start(wph_sb, wph_r)
            nc.sync.dma_start(wpl_sb, wpl_r)

            epool = ctx.enter_context(tc.tile_pool(name="epool", bufs=6))
            ytp_pool = ctx.enter_context(tc.tile_pool(name="ytp", bufs=2))
            rpool = ctx.enter_context(tc.tile_pool(name="rpool", bufs=4))
            spool = ctx.enter_context(tc.tile_pool(name="spool", bufs=4))
            ps_big = ctx.enter_context(tc.tile_pool(name="psbig", bufs=2, space="PSUM"))
            ps_s = ctx.enter_context(tc.tile_pool(name="pss", bufs=3, space="PSUM"))
            ps_y = ctx.enter_context(tc.tile_pool(name="psy", bufs=2, space="PSUM"))
            ps_t = ctx.enter_context(tc.tile_pool(name="pst", bufs=1, space="PSUM"))

            # ---- deficit-based filler scheduler (generator-chunked) ----
            # Fillers are generators yielding every ~3 matmuls (~300-400ns of
            # PE work) so drain() can match the ~250ns/block Act-PE imbalance
            # without starving either engine.
            sched = {"d": 0.0}
            fq = []          # list of dicts: key, fn (generator factory)
            cur = {"g": None, "key": None}

            def dr3(ps, lhs_h, lhs_l, rhs_h, rhs_l):
                """3-term fp8 DoubleRow GEMM over 1024-deep contraction;
                yields every 3 matmuls."""
                terms = ((lhs_h, rhs_h), (lhs_l, rhs_h), (lhs_h, rhs_l))
                n = 0
                for ti, (lt, rt) in enumerate(terms):
                    for a4 in range(4):
                        sl = slice(2 * a4, 2 * a4 + 2)
                        nc.tensor.matmul(
                            ps, lhsT=lt[:, sl, :], rhs=rt[:, sl, :],
                            start=(ti == 0 and a4 == 0),
                            stop=(ti == 2 and a4 == 3),
                            perf_mode=DR)
                        sched["d"] -= 256 * PE_NS
                        n += 1
                        if n % 3 == 0:
                            yield

            def qk_group(jt, i):
                t0 = i * QT
                c = slice(jt * 128, (jt + 1) * 128)
                s = slice(t0, t0 + QT)
                ps = ps_big.tile([128, QT], f32, name="pqk", tag="big")
                yield from dr3(ps, wqkh_sb[:, :, c], wqkl_sb[:, :, c],
                               xh_sb[:, :, s], xl_sb[:, :, s])
                nc.vector.tensor_copy(qk_sb[:, jt, s], ps)

            def v_group(tt, i):
                tg = 4 * i + tt
                s = slice(tg * 128, (tg + 1) * 128)
                ps = ps_big.tile([128, H_LOC * HD], f32, name="pv", tag="big")
                yield from dr3(ps, xh_sb[:, :, s], xl_sb[:, :, s], wvh_sb, wvl_sb)
                nc.vector.tensor_copy(
                    vt_sb[:, tg, :, 0:HD], ps.rearrange("p (h d) -> p h d", d=HD))

            def tp_p4(tg):
                yflat = y_tiles[tg].rearrange("p h d -> p (h d)")
                tp = ps_t.tile([128, 4, 128], b16, name="tp", tag="tp")
                for jc in range(4):
                    nc.tensor.transpose(
                        tp[:, jc, :], yflat[:, jc * 128:(jc + 1) * 128], id_sb)
                sched["d"] -= 512 * PE_NS
                yth = ytp_pool.tile([128, 4, 128], fp8, name="yth", tag="yth")
                ytl = ytp_pool.tile([128, 4, 128], fp8, name="ytl", tag="ytl")
                nc.vector.tensor_scalar_mul(yth, tp, SY)
                nc.vector.scalar_tensor_tensor(
                    ytl, tp, SY, yth, mybir.AluOpType.mult, mybir.AluOpType.subtract)
                yield
                for ot in range(2):
                    so = slice(ot * 512, (ot + 1) * 512)
                    ps = ps_big.tile([128, 512], f32, name="po", tag="big")
                    terms = ((yth, wph_sb), (ytl, wph_sb), (yth, wpl_sb))
                    n = 0
                    for ti, (lt, rt) in enumerate(terms):
                        for c4 in range(2):
                            sl = slice(2 * c4, 2 * c4 + 2)
                            nc.tensor.matmul(
                                ps, lhsT=lt[:, sl, :], rhs=rt[:, sl, so],
                                start=(ti == 0 and c4 == 0),
                                stop=(ti == 2 and c4 == 1),
                                perf_mode=DR)
                            sched["d"] -= 256 * PE_NS
                            n += 1
                            if n % 3 == 0:
                                yield
                    st = spool.tile([128, 512], f32, name="st", tag="st")
                    nc.vector.tensor_scalar_mul(st, ps, 1.0 / (SY * SW))
                    nc.sync.dma_start(
                        out_d[tg * 128:(tg + 1) * 128, so], st)

            def _advance():
                """Run one chunk of the current/next filler. False if empty."""
                if cur["g"] is None:
                    if not fq:
                        return False
                    ent = fq.pop(0)
                    cur["g"] = ent["fn"]()
                    cur["key"] = ent["key"]
                try:
                    next(cur["g"])
                except StopIteration:
                    cur["g"] = None
                    cur["key"] = None
                return True

            def drain(thresh=0.0):
                while sched["d"] > thresh:
                    if not _advance():
                        return

            def _exhaust(gen):
                for _ in gen:
                    pass

            def ensure(key):
                if cur["key"] == key:
                    _exhaust(cur["g"])
                    cur["g"] = None
                    cur["key"] = None
                    return
                for idx, ent in enumerate(fq):
                    if ent["key"] == key:
                        _exhaust(fq.pop(idx)["fn"]())
                        return

            def attn_head(i, p, hh):
                h = 2 * p + hh
                kt_hi = 4 * i + 3
                q_ap = qk_sb[hh * 64:(hh + 1) * 64, p, :]
                k_ap = qk_sb[hh * 64:(hh + 1) * 64, 4 + p, :]
                y_ps = ps_y.tile([128, 4, 66], f32, name="yps", tag="yps")
                pend = None   # (kt, e, r)

                def emit_y(kt, e, r):
                    for sub in range(max(0, r), 4):
                        nc.tensor.matmul(
                            y_ps[:, sub, 0:HD + 1],
                            lhsT=e[:, sub * 128:(sub + 1) * 128],
                            rhs=vt_sb[:, kt, h, 0:HD + 1],
                            start=(kt == 0 and sub == max(0, r)),
                            stop=(kt == kt_hi and sub == 3))
                    sched["d"] -= (4 - max(0, r)) * 65 * PE_NS

                for kt in range(kt_hi + 1):
                    r = kt - 4 * i
                    c0 = 128 * r if r > 0 else 0
                    s_ps = ps_s.tile([128, QT], f32, name="sps", tag="sps")
                    nc.tensor.matmul(
                        s_ps[:, c0:], lhsT=k_ap[:, kt * 128:(kt + 1) * 128],
                        rhs=q_ap[:, i * QT + c0:(i + 1) * QT],
                        start=True, stop=True)
                    sched["d"] -= (QT - c0) * PE_NS
                    e = epool.tile([128, QT], b16, name="e", tag="e")
                    nc.scalar.activation(
                        e[:, c0:], s_ps[:, c0:], EXP,
                        scale=0.125 / (SX * SW) ** 2)
                    sched["d"] += (QT - c0) * ACT_NS + ACT_OVH
                    if r >= 0:
                        # causal mask on the idle Pool engine (keeps the DVE
                        # queue free of Act-paced ops): keep where q >= k
                        nc.gpsimd.affine_select(
                            e[:, c0:c0 + 128], e[:, c0:c0 + 128],
                            pattern=[[1, 128]],
                            compare_op=mybir.AluOpType.is_ge,
                            fill=0.0, base=0, channel_multiplier=-1)
                    if pend is not None:
                        emit_y(*pend)
                    pend = (kt, e, r)
                    drain()
                emit_y(*pend)
                rc = rpool.tile([128, 4], f32, name="rc", tag="rc")
                nc.vector.reciprocal(rc, y_ps[:, :, HD])
                for sub in range(4):
                    tg = 4 * i + sub
                    nc.vector.tensor_scalar_mul(
                        y_tiles[tg][:, h, :], y_ps[:, sub, 0:HD], rc[:, sub:sub + 1])

            # ---- emission ----
            # queue all future proj groups as fillers (in dependency-safe order)
            for i in range(NST):
                for jt in (0, 4, 1, 5, 2, 6, 3, 7):
                    fq.append({"key": ("qk", i, jt),
                               "fn": (lambda jt=jt, i=i: qk_group(jt, i))})
                for tt in range(4):
                    fq.append({"key": ("v", i, tt),
                               "fn": (lambda tt=tt, i=i: v_group(tt, i))})

            for i in range(NST):
                for p in range(4):
                    ensure(("qk", i, p))
                    ensure(("qk", i, 4 + p))
                    if p == 0:
                        for tt in range(4):
                            ensure(("v", i, tt))
                    for hh in range(2):
                        attn_head(i, p, hh)
                        drain()
                # step's tp/P4 becomes deferred filler
                for tt in range(4):
                    fq.append({"key": ("p4", i, tt),
                               "fn": (lambda tg=4 * i + tt: tp_p4(tg))})
            if cur["g"] is not None:
                _exhaust(cur["g"])
            while fq:
                _exhaust(fq.pop(0)["fn"]())

    nc.compile()
    return nc


def _split8(m):
    hi = m.astype(f8)
    lo = (m - hi.astype(np.float32)).astype(f8)
    return hi, lo


def _prep_inputs(x, w_attn, w_proj):
    # tri[kl, ql] = 1 if ql >= kl (keep), else 0 (causal-masked)
    tri = np.ascontiguousarray(np.triu(np.ones((128, 128), np.float32))).astype(bf16)
    ident = np.ascontiguousarray(np.eye(128, dtype=np.float32)).astype(bf16)
    in_maps = []
    for core in range(N_CORES):
        b, g = core // 2, core % 2
        heads = [8 * g + i for i in range(H_LOC)]
        q_rows = np.concatenate([w_attn[HD * h:HD * h + HD] for h in heads])
        k_rows = np.concatenate([w_attn[C + HD * h:C + HD * h + HD] for h in heads])
        v_rows = np.concatenate([w_attn[2 * C + HD * h:2 * C + HD * h + HD] for h in heads])
        wqk = np.ascontiguousarray(np.concatenate([q_rows, k_rows]).T)
        wv = np.ascontiguousarray(v_rows.T)
        wp = np.ascontiguousarray(
            np.concatenate([w_proj[:, HD * h:HD * h + HD] for h in heads], axis=1).T)
        xT = np.ascontiguousarray(x[b].T)
        xh, xl = _split8(xT * SX)
        wqkh, wqkl = _split8(wqk * SW)
        wvh, wvl = _split8(wv * SW)
        wph, wpl = _split8(wp * SW)
        in_maps.append({
            "xh": xh, "xl": xl, "wqkh": wqkh, "wqkl": wqkl,
            "wvh": wvh, "wvl": wvl, "wph": wph, "wpl": wpl,
            "tri": tri, "ident": ident})
    return in_maps


def kernel(x, w_attn, w_proj):
    from concourse.bass_utils import run_bass_kernel_spmd

    x = np.asarray(x, dtype=np.float32)
    w_attn = np.asarray(w_attn, dtype=np.float32)
    w_proj = np.asarray(w_proj, dtype=np.float32)

    if "nc" not in _CACHE:
        _CACHE["nc"] = _build()
    nc = _CACHE["nc"]

    in_maps = _prep_inputs(x, w_attn, w_proj)
    res = run_bass_kernel_spmd(nc, in_maps, core_ids=list(range(N_CORES)))
    outs = [res.results[c]["out"] for c in range(N_CORES)]
    y = np.stack([outs[2 * b] + outs[2 * b + 1] for b in range(B)])
    return y.astype(np.float32)
